# revision 44
# baseline (speedup 1.0000x reference)
"""Gaussian-splat tile renderer for one TRN2 chip (8 NeuronCores).

Host side: depth sort + per-tile gaussian binning (first K=64 overlapping
gaussians per 16x16 tile in depth order), then packing tiles into
128-partition "groups" (first-fit-decreasing over per-tile gaussian
counts) so the device never pays for empty K slots.

Device side (SPMD over 8 cores, 128 tiles each), per group of packed
tiles (partition axis = packed gaussian slots, free axis = 256 pixels):
  1. TensorE: Q = G12^T @ F12   (quadratic form exponent + ln(opacity),
     bf16 hi/lo split for fp32-grade accuracy at bf16 speed)
  2. ScalarE: alpha = exp(Q); VectorE: clip to [0.01, 0.99]  (fp16)
  3. ScalarE: lnb = ln(1 - alpha)
  4. TensorE: lnW = TRI_g^T @ lnb  (blocked strict-lower-triangular
     prefix-sum -> log transmittance, per-group block structure)
  5. ScalarE: W = exp(lnW); VectorE: aw = alpha * W  (fp16, 4x mode)
  6. TensorE: img_cols = aw^T @ col  (block-diag colors -> packed PSUM)
The three ScalarE LUT passes are emitted phase-contiguous and both Exp
and Ln are resolved to the one PWP table set that contains them both,
so the kernel performs a single ACT table load. Tiles are packed into
groups globally and dealt round-robin so all 8 cores get equal work.
"""

import os
import sys
import types
import tempfile

import numpy as np

N_GAUSS = 16384
IMG_W = 512
IMG_H = 512
TILE = 16
K_MAX = 64
TX = IMG_W // TILE   # 32
TY = IMG_H // TILE   # 32
T = TX * TY          # 1024
NCORES = 8
TPC = T // NCORES    # 128 tiles per core
NG = 56              # packed groups per core (padded; multiple of 4)

LAST_EXEC_TIME_NS = None
LAST_TRACE_PATH = None

_CACHED_NC = None


# ---------------------------------------------------------------- host prep

def _host_bin(pos2d, cov2d, opacity, color, depth):
    """Depth-sort + per-tile first-K selection. Returns [T, K] gathered
    params and validity."""
    t = TILE
    K = K_MAX

    a = cov2d[:, 0, 0]; b = cov2d[:, 0, 1]; c = cov2d[:, 1, 1]
    trace = a + c
    det = a * c - b * b
    term1 = 0.5 * trace
    term2 = 0.5 * np.sqrt(np.clip(trace * trace - 4.0 * det, 0.0, None))
    radius = 3.0 * np.sqrt(np.maximum(term1 - term2, term1 + term2))

    order = np.argsort(depth, kind='stable')
    pos2d = pos2d[order]; cov2d = cov2d[order]
    opacity = opacity[order]; color = color[order]; radius = radius[order]

    lefts = np.repeat(np.arange(TX) * t, TY).astype(np.float32)   # [T]
    tops = np.tile(np.arange(TY) * t, TX).astype(np.float32)      # [T]
    px = pos2d[None, :, 0]; py = pos2d[None, :, 1]; r = radius[None, :]
    L = lefts[:, None]; Tp = tops[:, None]
    overlap = (px + r > L) & (px - r < L + t) & (py + r > Tp) & (py - r < Tp + t)

    rank = np.cumsum(overlap, axis=1, dtype=np.int32)              # [T, N]
    counts = np.minimum(rank[:, -1], K)                            # [T]
    mask = overlap & (rank <= K)
    rows, cols = np.nonzero(mask)
    slot = rank[rows, cols] - 1
    sel = np.zeros((T, K), dtype=np.int64)
    sel[rows, slot] = cols
    valid = np.arange(K)[None, :] < counts[:, None]                # [T, K]

    tp = pos2d[sel]            # [T, K, 2]
    tcov = cov2d[sel]          # [T, K, 2, 2]
    topac = opacity[sel]       # [T, K]
    tcol = color[sel]          # [T, K, 3]
    return tp, tcov, topac, tcol, valid, counts, lefts, tops


def _pack_once(items, cap, max_tiles, best_fit):
    groups = []   # [used, [(tile, base, cnt), ...]]
    for (c, tl) in items:
        cand = None
        for gi, grp in enumerate(groups):
            if grp[0] + c <= cap and len(grp[1]) < max_tiles:
                if not best_fit:
                    cand = gi
                    break
                if cand is None or grp[0] > groups[cand][0]:
                    cand = gi
        if cand is None:
            groups.append([c, [(tl, 0, c)]])
        else:
            grp = groups[cand]
            grp[1].append((tl, grp[0], c))
            grp[0] += c
    return [g[1] for g in groups]


def _ffd_pack(counts_core, max_tiles=5):
    """Pack tiles (by gaussian count) into 128-slot groups (at most
    max_tiles tiles per group, matching the 16-column per-group output
    budget). Tries first-fit and best-fit decreasing, keeps the best.
    Returns list of groups; each group is a list of
    (tile_local_idx, base_slot, count)."""
    items = sorted(((int(c), int(tl)) for tl, c in enumerate(counts_core)
                    if c > 0), reverse=True)
    a = _pack_once(items, 128, max_tiles, best_fit=False)
    b = _pack_once(items, 128, max_tiles, best_fit=True)
    return a if len(a) <= len(b) else b


def _host_pack(tp, tcov, topac, tcol, valid, counts, lefts, tops):
    """Build per-core device inputs with FFD slot packing."""
    import ml_dtypes
    bf16 = ml_dtypes.bfloat16

    ga = tcov[:, :, 0, 0]; gb = tcov[:, :, 0, 1]; gc = tcov[:, :, 1, 1]
    gdet = ga * gc - gb * gb
    s = (-0.5 / gdet).astype(np.float32)
    X = tp[:, :, 0] - lefts[:, None]
    Y = tp[:, :, 1] - tops[:, None]
    lnop = np.log(np.maximum(topac, 1e-30)).astype(np.float32)

    G = np.empty((T, K_MAX, 6), np.float32)
    G[:, :, 0] = s * gc
    G[:, :, 1] = -2.0 * s * gb
    G[:, :, 2] = s * ga
    G[:, :, 3] = s * (-2.0 * gc * X + 2.0 * gb * Y)
    G[:, :, 4] = s * (2.0 * gb * X - 2.0 * ga * Y)
    G[:, :, 5] = s * (gc * X * X - 2.0 * gb * X * Y + ga * Y * Y) + lnop

    tcolv = np.where(valid[:, :, None], tcol, 0.0).astype(np.float32)

    # global pack over all tiles, then deal bins round-robin so every
    # core gets the same number of groups (the slowest core sets the
    # SPMD exec time)
    global NG
    gbins = _ffd_pack(counts)          # tile ids are global here
    order = np.argsort([-sum(c for (_, _, c) in g) for g in gbins])
    core_groups = [[] for _ in range(NCORES)]
    for bi, gi in enumerate(order):
        core_groups[bi % NCORES].append(gbins[gi])
    need = max(4, -(-max(len(g) for g in core_groups) // 4) * 4)
    if need > NG:   # unexpected data shape: grow the program
        NG = need

    G12s, TRIs, col6s, maps = [], [], [], []
    idx = np.arange(128)
    for core in range(NCORES):
        groups = core_groups[core]

        G6 = np.zeros((6, NG * 128), np.float32)
        G6[5, :] = -20.0
        TRI = np.zeros((128, NG * 128), np.float16)
        col6 = np.zeros((128, NG * 16), np.float32)
        amap = []   # (tile_global, group, index_in_group)
        for g, grp in enumerate(groups):
            for i, (tg, base, c) in enumerate(grp):
                sl = slice(g * 128 + base, g * 128 + base + c)
                G6[:, sl] = G[tg, :c].T
                TRI[base:base + c, g * 128 + base:g * 128 + base + c] = \
                    (idx[base:base + c, None] < idx[None, base:base + c])
                col6[base:base + c, 16 * g + 3 * i:16 * g + 3 * i + 3] = \
                    tcolv[tg, :c]
                amap.append((tg, g, i))
        Ghi = G6.astype(bf16)
        Glo = (G6 - Ghi.astype(np.float32)).astype(bf16)
        G12 = np.concatenate([Ghi, Glo], axis=0)        # [12, NG*128]
        # stack pairs of groups along the contraction dim: one matmul
        # computes 2 groups (512 psum cols) against blockdiag F24
        G24 = np.zeros((24, (NG // 2) * 128), bf16)
        G12v = G12.reshape(12, NG, 128)
        G24.reshape(2, 12, NG // 2, 128)[0] = G12v[:, 0::2]
        G24.reshape(2, 12, NG // 2, 128)[1] = G12v[:, 1::2]
        G12s.append(np.ascontiguousarray(G24))
        TRIs.append(np.ascontiguousarray(TRI))
        col6s.append(np.ascontiguousarray(col6.astype(np.float16)))
        maps.append(amap)

    u = (np.arange(256) // 16).astype(np.float32)
    v = (np.arange(256) % 16).astype(np.float32)
    F = np.stack([u * u, u * v, v * v, u, v, np.ones(256, np.float32)])
    F12 = np.concatenate([F, F], axis=0).astype(np.float32)  # [12, 256]
    F24 = np.zeros((24, 512), np.float32)
    F24[0:12, 0:256] = F12
    F24[12:24, 256:512] = F12
    F24 = np.ascontiguousarray(F24.astype(bf16))

    return G12s, TRIs, col6s, F24, maps


# ------------------------------------------------------------- device build

def _pin_act_table_set():
    """Make bacc's table-load pass resolve both Exp and Ln to the one
    PWP set that contains them both (natural_log_exp_and_others), so the
    kernel needs a single ACT_TABLE_LOAD instead of one per Exp<->Ln
    transition. Set ids are indices into act_info.json, so entries are
    filtered in place rather than removed."""
    import concourse.mybir as mybir
    import concourse.hw_specs as hw_specs
    import concourse.bacc as bacc

    orig = hw_specs.get_activation_tables
    if getattr(orig, "_gsplat_pinned", False):
        return

    def patched(module_arch):
        tables = orig(module_arch)
        exp, ln = (mybir.ActivationFunctionType.Exp,
                   mybir.ActivationFunctionType.Ln)
        both = next((n for n, fs in tables.items()
                     if exp in fs and ln in fs), None)
        if both is not None:
            for name, fs in tables.items():
                if name != both:
                    fs.discard(exp)
                    fs.discard(ln)
        return tables

    patched._gsplat_pinned = True
    hw_specs.get_activation_tables = patched
    if getattr(bacc, "get_activation_tables", None) is not None:
        bacc.get_activation_tables = patched


def _build_nc():
    import concourse.bacc as bacc
    import concourse.mybir as mybir
    import concourse.tile as tile

    _pin_act_table_set()

    f32 = mybir.dt.float32
    bf16 = mybir.dt.bfloat16
    fp16 = mybir.dt.float16
    Alu = mybir.AluOpType
    Act = mybir.ActivationFunctionType

    nc = bacc.Bacc("TRN2", target_bir_lowering=False, debug=False,
                   num_devices=NCORES)
    g12_d = nc.dram_tensor("g12", [24, (NG // 2) * 128], bf16,
                           kind="ExternalInput")
    f12_d = nc.dram_tensor("f12", [24, 512], bf16, kind="ExternalInput")
    tri_d = nc.dram_tensor("tri", [128, NG * 128], fp16, kind="ExternalInput")
    col6_d = nc.dram_tensor("col6", [128, NG * 16], fp16, kind="ExternalInput")
    out_d = nc.dram_tensor("out", [128, NG * 32], f32, kind="ExternalOutput")

    with tile.TileContext(nc) as tc:
        with (
            tc.tile_pool(name="const", bufs=1) as cpool,
            tc.tile_pool(name="sb", bufs=1) as slab,
            tc.tile_pool(name="tmp", bufs=6) as tmp,
            tc.tile_pool(name="qw", bufs=3, space="PSUM") as qw,
            tc.tile_pool(name="cp", bufs=1, space="PSUM") as cp,
        ):
            g12_s = cpool.tile([24, (NG // 2) * 128], bf16, tag="g12")
            f12_s = cpool.tile([24, 512], bf16, tag="f12")
            # small first chunk so the first quad matmuls (and the first
            # scalar-engine exp) can start as early as possible; the
            # remainder streams on a different DMA queue in parallel
            nc.sync.dma_start(f12_s[:], f12_d[:])
            nc.sync.dma_start(g12_s[:, 0:512], g12_d[:, 0:512])
            nc.gpsimd.dma_start(g12_s[:, 512:(NG // 2) * 128],
                                g12_d[:, 512:(NG // 2) * 128])
            tri_s = cpool.tile([128, NG * 128], fp16, tag="tri")
            col6_s = cpool.tile([128, NG * 16], fp16, tag="col6")

            alpha = slab.tile([128, NG * 256], fp16, tag="alpha")
            lnb = slab.tile([128, NG * 256], fp16, tag="lnb")
            outs = slab.tile([128, NG * 32], f32, tag="outs")

            # ---- phase A: quad matmuls + exp + clip --------------------
            for qi in range(NG // 4):
                q = qw.tile([128, 1024], f32, tag="qw")
                for p2 in range(2):
                    gp = 2 * qi + p2   # stacked pair = groups (4qi+2p2, +1)
                    nc.tensor.matmul(
                        q[:, 512 * p2: 512 * p2 + 512],
                        g12_s[:, 128 * gp: 128 * gp + 128],
                        f12_s[:],
                    )
                au = tmp.tile([128, 1024], fp16, tag="au")
                nc.scalar.activation(au[:], q[:], Act.Exp)
                nc.vector.tensor_scalar(
                    alpha[:, 1024 * qi: 1024 * qi + 1024],
                    au[:], 0.01, 0.99, Alu.max, Alu.min)

            # constants for later phases stream in during phase A, on the
            # gpsimd DMA queue so they don't delay the phase-A inputs
            half = NG * 64
            nc.gpsimd.dma_start(tri_s[:, 0:half], tri_d[:, 0:half])
            nc.gpsimd.dma_start(tri_s[:, half:2 * half],
                                tri_d[:, half:2 * half])
            nc.gpsimd.dma_start(col6_s[:], col6_d[:])

            # ---- phase B: ln(1 - alpha) --------------------------------
            for li in range(2):
                sl = slice((NG * 128) * li, (NG * 128) * (li + 1))
                nc.scalar.activation(lnb[:, sl], alpha[:, sl], Act.Ln,
                                     bias=1.0, scale=-1.0)

            # ---- phase C: prefix matmuls, exp, alpha*W, color ----------
            # color PSUM: two chunks per pixel-half (16 cols/group;
            # chunk sizes are multiples of 4 groups, <= 32 per bank)
            ch0 = min(32, 4 * -(-NG // 8))       # e.g. NG=52 -> 28
            # a tiny final chunk keeps the post-compute tail (copy+DMA
            # after the very last matmul) as short as possible
            chunks = ((0, ch0), (ch0, NG - ch0 - 4), (NG - 4, 4))
            for (cbase, csz) in chunks:
                cp0 = cp.tile([128, csz * 16], f32, tag="c0")
                cp1 = cp.tile([128, csz * 16], f32, tag="c1")
                cps = (cp0, cp1)
                for wi in range(csz // 4):
                    w = qw.tile([128, 1024], f32, tag="qw")
                    for p4 in range(4):
                        g = cbase + 4 * wi + p4
                        nc.tensor.matmul(
                            w[:, 256 * p4: 256 * p4 + 256],
                            tri_s[:, 128 * g: 128 * g + 128],
                            lnb[:, 256 * g: 256 * g + 256],
                        )
                    g0 = cbase + 4 * wi
                    wS = tmp.tile([128, 1024], fp16, tag="ws")
                    nc.scalar.activation(wS[:], w[:], Act.Exp)
                    aw = tmp.tile([128, 1024], fp16, tag="aw")
                    nc.vector.tensor_tensor(
                        aw[:], alpha[:, 256 * g0: 256 * g0 + 1024], wS[:],
                        op=Alu.mult)
                    for p4 in range(4):
                        g = g0 + p4
                        for h in range(2):
                            nc.tensor.matmul(
                                cps[h][:, 16 * (g - cbase):
                                       16 * (g - cbase) + 16],
                                aw[:, 256 * p4 + 128 * h:
                                   256 * p4 + 128 * h + 128],
                                col6_s[:, 16 * g: 16 * g + 16],
                            )
                o0 = slice(cbase * 16, (cbase + csz) * 16)
                o1 = slice(NG * 16 + cbase * 16, NG * 16 + (cbase + csz) * 16)
                nc.vector.tensor_copy(outs[:, o0], cp0[:])
                nc.vector.tensor_copy(outs[:, o1], cp1[:])
                nc.sync.dma_start(out_d[:, o0], outs[:, o0])
                nc.gpsimd.dma_start(out_d[:, o1], outs[:, o1])

    nc.compile()
    return nc


def _ensure_ntff_hook():
    try:
        from antenv.axon_hooks import get_axon_ntff_profile_hook  # noqa: F401
        import antenv.axon_hooks as ah
    except ImportError:
        import antenv
        mod = types.ModuleType("antenv.axon_hooks")
        mod._hook = None
        def _set(h):
            mod._hook = h
        def _get():
            return mod._hook
        mod.set_axon_ntff_profile_hook = _set
        mod.get_axon_ntff_profile_hook = _get
        sys.modules["antenv.axon_hooks"] = mod
        antenv.axon_hooks = mod
        ah = mod
    if ah.get_axon_ntff_profile_hook() is None:
        from trn_agent_boot.trn_boot import _ntff_profile_via_ctypes
        ah.set_axon_ntff_profile_hook(
            _ntff_profile_via_ctypes('/opt/axon/libaxon_pjrt.so'))


# ------------------------------------------------------- numpy fallback path

def _render_numpy(pos2d, cov2d, opacity, color, depth):
    tp, tcov, topac, tcol, valid, counts, lefts, tops = _host_bin(
        pos2d, cov2d, opacity, color, depth)
    t = TILE
    gi, gj = np.meshgrid(np.arange(t), np.arange(t), indexing='ij')
    base = np.stack([gi, gj], axis=-1).astype(np.float32)
    offs = np.stack([lefts, tops], axis=-1)
    pix = base[None] + offs[:, None, None, :]
    dx = pix[:, :, :, None, 0] - tp[:, None, None, :, 0]
    dy = pix[:, :, :, None, 1] - tp[:, None, None, :, 1]
    ga = tcov[:, :, 0, 0][:, None, None, :]
    gb = tcov[:, :, 0, 1][:, None, None, :]
    gc = tcov[:, :, 1, 1][:, None, None, :]
    gdet = ga * gc - gb * gb
    quad = (gc * dx * dx - 2.0 * gb * dx * dy + ga * dy * dy) / gdet
    prob = np.exp(-0.5 * quad)
    alpha = np.clip(topac[:, None, None, :] * prob, 0.01, 0.99)
    alpha = np.where(valid[:, None, None, :], alpha, 0.0)
    weight = np.empty_like(alpha)
    weight[..., 0] = 1.0
    np.cumprod(1.0 - alpha[..., :-1], axis=-1, out=weight[..., 1:])
    aw = (alpha * weight).reshape(T, t * t, K_MAX)
    tile_img = np.matmul(aw, tcol).reshape(T, t, t, 3)
    img = tile_img.reshape(TX, TY, t, t, 3).transpose(0, 2, 1, 3, 4)
    return img.reshape(IMG_W, IMG_H, 3).astype(np.float32)


# ----------------------------------------------------------------- kernel()

def _run_device(pos2d, cov2d, opacity, color, depth):
    global _CACHED_NC, LAST_EXEC_TIME_NS, LAST_TRACE_PATH
    from concourse import bass_utils

    trace = bool(os.environ.get("GSPLAT_TRACE"))
    if trace:
        _ensure_ntff_hook()
        bass_utils.upload_artifacts = lambda tmpdir: "local://" + tmpdir

    tp, tcov, topac, tcol, valid, counts, lefts, tops = _host_bin(
        pos2d, cov2d, opacity, color, depth)
    G12s, TRIs, col6s, F12, maps = _host_pack(
        tp, tcov, topac, tcol, valid, counts, lefts, tops)

    if _CACHED_NC is None or _CACHED_NC[0] != NG:
        _CACHED_NC = (NG, _build_nc())
    nc = _CACHED_NC[1]

    in_maps = []
    for i in range(NCORES):
        in_maps.append({
            "g12": G12s[i],
            "f12": F12,
            "tri": TRIs[i],
            "col6": col6s[i],
        })
    kwargs = {}
    if trace:
        kwargs["trace"] = True
        kwargs["tmpdir"] = tempfile.mkdtemp(prefix="gsplat_trace_")
    res = bass_utils.run_bass_kernel_spmd(
        nc, in_maps, core_ids=list(range(NCORES)), **kwargs)
    if trace:
        LAST_EXEC_TIME_NS = res.exec_time_ns
        LAST_TRACE_PATH = (res.instructions_and_trace[1]
                           if res.instructions_and_trace else None)

    img_tiles = np.zeros((T, 256, 3), np.float32)
    for i in range(NCORES):
        o = res.results[i]["out"]                 # [128, NG*32]
        o = o.reshape(128, 2, NG, 16)             # [pixh, h, g, col]
        for (tg, g, ig) in maps[i]:
            t3 = o[:, :, g, 3 * ig:3 * ig + 3]    # [pixh, h, c]
            img_tiles[tg] = t3.transpose(1, 0, 2).reshape(256, 3)
    img = img_tiles.reshape(TX, TY, TILE, TILE, 3).transpose(0, 2, 1, 3, 4)
    return np.ascontiguousarray(
        img.reshape(IMG_W, IMG_H, 3), dtype=np.float32)


def kernel(pos2d, cov2d, opacity, color, depth, width=IMG_W, height=IMG_H,
           tile_length=TILE, max_per_tile=K_MAX):
    pos2d = np.asarray(pos2d, np.float32)
    cov2d = np.asarray(cov2d, np.float32)
    opacity = np.asarray(opacity, np.float32)
    color = np.asarray(color, np.float32)
    depth = np.asarray(depth, np.float32)
    ok = (int(width) == IMG_W and int(height) == IMG_H
          and int(tile_length) == TILE and int(max_per_tile) == K_MAX
          and pos2d.shape == (N_GAUSS, 2))
    if ok:
        try:
            return _run_device(pos2d, cov2d, opacity, color, depth)
        except Exception as e:  # pragma: no cover - device fallback
            print(f"kernel: device path failed ({type(e).__name__}: {e}); "
                  f"falling back to numpy", file=sys.stderr)
    return _render_numpy(pos2d, cov2d, opacity, color, depth)


# revision 45
# speedup vs baseline: 1.0015x; 1.0015x over previous
"""Gaussian-splat tile renderer for one TRN2 chip (8 NeuronCores).

Host side: depth sort + per-tile gaussian binning (first K=64 overlapping
gaussians per 16x16 tile in depth order), then packing tiles into
128-partition "groups" (first-fit-decreasing over per-tile gaussian
counts) so the device never pays for empty K slots.

Device side (SPMD over 8 cores, 128 tiles each), per group of packed
tiles (partition axis = packed gaussian slots, free axis = 256 pixels):
  1. TensorE: Q = G12^T @ F12   (quadratic form exponent + ln(opacity),
     bf16 hi/lo split for fp32-grade accuracy at bf16 speed)
  2. ScalarE: alpha = exp(Q); VectorE: clip to [0.01, 0.99]  (fp16)
  3. ScalarE: lnb = ln(1 - alpha)
  4. TensorE: lnW = TRI_g^T @ lnb  (blocked strict-lower-triangular
     prefix-sum -> log transmittance, per-group block structure)
  5. ScalarE: W = exp(lnW); VectorE: aw = alpha * W  (fp16, 4x mode)
  6. TensorE: img_cols = aw^T @ col  (block-diag colors -> packed PSUM)
The three ScalarE LUT passes are emitted phase-contiguous and both Exp
and Ln are resolved to the one PWP table set that contains them both,
so the kernel performs a single ACT table load. Tiles are packed into
groups globally and dealt round-robin so all 8 cores get equal work.
"""

import os
import sys
import types
import tempfile

import numpy as np

N_GAUSS = 16384
IMG_W = 512
IMG_H = 512
TILE = 16
K_MAX = 64
TX = IMG_W // TILE   # 32
TY = IMG_H // TILE   # 32
T = TX * TY          # 1024
NCORES = 8
TPC = T // NCORES    # 128 tiles per core
NG = 56              # packed groups per core (padded; multiple of 4)

LAST_EXEC_TIME_NS = None
LAST_TRACE_PATH = None

_CACHED_NC = None


# ---------------------------------------------------------------- host prep

def _host_bin(pos2d, cov2d, opacity, color, depth):
    """Depth-sort + per-tile first-K selection. Returns [T, K] gathered
    params and validity."""
    t = TILE
    K = K_MAX

    a = cov2d[:, 0, 0]; b = cov2d[:, 0, 1]; c = cov2d[:, 1, 1]
    trace = a + c
    det = a * c - b * b
    term1 = 0.5 * trace
    term2 = 0.5 * np.sqrt(np.clip(trace * trace - 4.0 * det, 0.0, None))
    radius = 3.0 * np.sqrt(np.maximum(term1 - term2, term1 + term2))

    order = np.argsort(depth, kind='stable')
    pos2d = pos2d[order]; cov2d = cov2d[order]
    opacity = opacity[order]; color = color[order]; radius = radius[order]

    lefts = np.repeat(np.arange(TX) * t, TY).astype(np.float32)   # [T]
    tops = np.tile(np.arange(TY) * t, TX).astype(np.float32)      # [T]
    px = pos2d[None, :, 0]; py = pos2d[None, :, 1]; r = radius[None, :]
    L = lefts[:, None]; Tp = tops[:, None]
    overlap = (px + r > L) & (px - r < L + t) & (py + r > Tp) & (py - r < Tp + t)

    rank = np.cumsum(overlap, axis=1, dtype=np.int32)              # [T, N]
    counts = np.minimum(rank[:, -1], K)                            # [T]
    mask = overlap & (rank <= K)
    rows, cols = np.nonzero(mask)
    slot = rank[rows, cols] - 1
    sel = np.zeros((T, K), dtype=np.int64)
    sel[rows, slot] = cols
    valid = np.arange(K)[None, :] < counts[:, None]                # [T, K]

    tp = pos2d[sel]            # [T, K, 2]
    tcov = cov2d[sel]          # [T, K, 2, 2]
    topac = opacity[sel]       # [T, K]
    tcol = color[sel]          # [T, K, 3]
    return tp, tcov, topac, tcol, valid, counts, lefts, tops


def _pack_once(items, cap, max_tiles, best_fit):
    groups = []   # [used, [(tile, base, cnt), ...]]
    for (c, tl) in items:
        cand = None
        for gi, grp in enumerate(groups):
            if grp[0] + c <= cap and len(grp[1]) < max_tiles:
                if not best_fit:
                    cand = gi
                    break
                if cand is None or grp[0] > groups[cand][0]:
                    cand = gi
        if cand is None:
            groups.append([c, [(tl, 0, c)]])
        else:
            grp = groups[cand]
            grp[1].append((tl, grp[0], c))
            grp[0] += c
    return [g[1] for g in groups]


def _ffd_pack(counts_core, max_tiles=5):
    """Pack tiles (by gaussian count) into 128-slot groups (at most
    max_tiles tiles per group, matching the 16-column per-group output
    budget). Tries first-fit and best-fit decreasing, keeps the best.
    Returns list of groups; each group is a list of
    (tile_local_idx, base_slot, count)."""
    items = sorted(((int(c), int(tl)) for tl, c in enumerate(counts_core)
                    if c > 0), reverse=True)
    a = _pack_once(items, 128, max_tiles, best_fit=False)
    b = _pack_once(items, 128, max_tiles, best_fit=True)
    return a if len(a) <= len(b) else b


def _host_pack(tp, tcov, topac, tcol, valid, counts, lefts, tops):
    """Build per-core device inputs with FFD slot packing."""
    import ml_dtypes
    bf16 = ml_dtypes.bfloat16

    ga = tcov[:, :, 0, 0]; gb = tcov[:, :, 0, 1]; gc = tcov[:, :, 1, 1]
    gdet = ga * gc - gb * gb
    s = (-0.5 / gdet).astype(np.float32)
    X = tp[:, :, 0] - lefts[:, None]
    Y = tp[:, :, 1] - tops[:, None]
    lnop = np.log(np.maximum(topac, 1e-30)).astype(np.float32)

    G = np.empty((T, K_MAX, 6), np.float32)
    G[:, :, 0] = s * gc
    G[:, :, 1] = -2.0 * s * gb
    G[:, :, 2] = s * ga
    G[:, :, 3] = s * (-2.0 * gc * X + 2.0 * gb * Y)
    G[:, :, 4] = s * (2.0 * gb * X - 2.0 * ga * Y)
    G[:, :, 5] = s * (gc * X * X - 2.0 * gb * X * Y + ga * Y * Y) + lnop

    tcolv = np.where(valid[:, :, None], tcol, 0.0).astype(np.float32)

    # global pack over all tiles, then deal bins round-robin so every
    # core gets the same number of groups (the slowest core sets the
    # SPMD exec time)
    global NG
    gbins = _ffd_pack(counts)          # tile ids are global here
    order = np.argsort([-sum(c for (_, _, c) in g) for g in gbins])
    core_groups = [[] for _ in range(NCORES)]
    for bi, gi in enumerate(order):
        core_groups[bi % NCORES].append(gbins[gi])
    need = max(4, -(-max(len(g) for g in core_groups) // 4) * 4)
    if need > NG:   # unexpected data shape: grow the program
        NG = need

    G12s, TRIs, col6s, maps = [], [], [], []
    idx = np.arange(128)
    for core in range(NCORES):
        groups = core_groups[core]

        G6 = np.zeros((6, NG * 128), np.float32)
        G6[5, :] = -20.0
        TRI = np.zeros((128, NG * 128), np.float16)
        col6 = np.zeros((128, NG * 16), np.float32)
        amap = []   # (tile_global, group, index_in_group)
        for g, grp in enumerate(groups):
            for i, (tg, base, c) in enumerate(grp):
                sl = slice(g * 128 + base, g * 128 + base + c)
                G6[:, sl] = G[tg, :c].T
                TRI[base:base + c, g * 128 + base:g * 128 + base + c] = \
                    (idx[base:base + c, None] < idx[None, base:base + c])
                col6[base:base + c, 16 * g + 3 * i:16 * g + 3 * i + 3] = \
                    tcolv[tg, :c]
                amap.append((tg, g, i))
        Ghi = G6.astype(bf16)
        Glo = (G6 - Ghi.astype(np.float32)).astype(bf16)
        G12 = np.concatenate([Ghi, Glo], axis=0)        # [12, NG*128]
        # stack pairs of groups along the contraction dim: one matmul
        # computes 2 groups (512 psum cols) against blockdiag F24
        G24 = np.zeros((24, (NG // 2) * 128), bf16)
        G12v = G12.reshape(12, NG, 128)
        G24.reshape(2, 12, NG // 2, 128)[0] = G12v[:, 0::2]
        G24.reshape(2, 12, NG // 2, 128)[1] = G12v[:, 1::2]
        G12s.append(np.ascontiguousarray(G24))
        TRIs.append(np.ascontiguousarray(TRI))
        col6s.append(np.ascontiguousarray(col6.astype(np.float16)))
        maps.append(amap)

    u = (np.arange(256) // 16).astype(np.float32)
    v = (np.arange(256) % 16).astype(np.float32)
    F = np.stack([u * u, u * v, v * v, u, v, np.ones(256, np.float32)])
    F12 = np.concatenate([F, F], axis=0).astype(np.float32)  # [12, 256]
    F24 = np.zeros((24, 512), np.float32)
    F24[0:12, 0:256] = F12
    F24[12:24, 256:512] = F12
    F24 = np.ascontiguousarray(F24.astype(bf16))

    return G12s, TRIs, col6s, F24, maps


# ------------------------------------------------------------- device build

def _pin_act_table_set():
    """Make bacc's table-load pass resolve both Exp and Ln to the one
    PWP set that contains them both (natural_log_exp_and_others), so the
    kernel needs a single ACT_TABLE_LOAD instead of one per Exp<->Ln
    transition. Set ids are indices into act_info.json, so entries are
    filtered in place rather than removed."""
    import concourse.mybir as mybir
    import concourse.hw_specs as hw_specs
    import concourse.bacc as bacc

    orig = hw_specs.get_activation_tables
    if getattr(orig, "_gsplat_pinned", False):
        return

    def patched(module_arch):
        tables = orig(module_arch)
        exp, ln = (mybir.ActivationFunctionType.Exp,
                   mybir.ActivationFunctionType.Ln)
        both = next((n for n, fs in tables.items()
                     if exp in fs and ln in fs), None)
        if both is not None:
            for name, fs in tables.items():
                if name != both:
                    fs.discard(exp)
                    fs.discard(ln)
        return tables

    patched._gsplat_pinned = True
    hw_specs.get_activation_tables = patched
    if getattr(bacc, "get_activation_tables", None) is not None:
        bacc.get_activation_tables = patched


def _build_nc():
    import concourse.bacc as bacc
    import concourse.mybir as mybir
    import concourse.tile as tile

    _pin_act_table_set()

    f32 = mybir.dt.float32
    bf16 = mybir.dt.bfloat16
    fp16 = mybir.dt.float16
    Alu = mybir.AluOpType
    Act = mybir.ActivationFunctionType

    nc = bacc.Bacc("TRN2", target_bir_lowering=False, debug=False,
                   num_devices=NCORES)
    g12_d = nc.dram_tensor("g12", [24, (NG // 2) * 128], bf16,
                           kind="ExternalInput")
    f12_d = nc.dram_tensor("f12", [24, 512], bf16, kind="ExternalInput")
    tri_d = nc.dram_tensor("tri", [128, NG * 128], fp16, kind="ExternalInput")
    col6_d = nc.dram_tensor("col6", [128, NG * 16], fp16, kind="ExternalInput")
    out_d = nc.dram_tensor("out", [128, NG * 32], f32, kind="ExternalOutput")

    with tile.TileContext(nc) as tc:
        with (
            tc.tile_pool(name="const", bufs=1) as cpool,
            tc.tile_pool(name="sb", bufs=1) as slab,
            tc.tile_pool(name="tmp", bufs=4) as tmp,
            tc.tile_pool(name="qw", bufs=3, space="PSUM") as qw,
            tc.tile_pool(name="cp", bufs=1, space="PSUM") as cp,
        ):
            g12_s = cpool.tile([24, (NG // 2) * 128], bf16, tag="g12")
            f12_s = cpool.tile([24, 512], bf16, tag="f12")
            # small first chunk so the first quad matmuls (and the first
            # scalar-engine exp) can start as early as possible; the
            # remainder streams on a different DMA queue in parallel
            nc.sync.dma_start(f12_s[:], f12_d[:])
            nc.sync.dma_start(g12_s[:, 0:512], g12_d[:, 0:512])
            nc.gpsimd.dma_start(g12_s[:, 512:(NG // 2) * 128],
                                g12_d[:, 512:(NG // 2) * 128])
            tri_s = cpool.tile([128, NG * 128], fp16, tag="tri")
            col6_s = cpool.tile([128, NG * 16], fp16, tag="col6")

            alpha = slab.tile([128, NG * 256], fp16, tag="alpha")
            lnb = slab.tile([128, NG * 256], fp16, tag="lnb")
            outs = slab.tile([128, NG * 32], f32, tag="outs")

            # ---- phase A: quad matmuls + exp + clip --------------------
            for qi in range(NG // 4):
                q = qw.tile([128, 1024], f32, tag="qw")
                for p2 in range(2):
                    gp = 2 * qi + p2   # stacked pair = groups (4qi+2p2, +1)
                    nc.tensor.matmul(
                        q[:, 512 * p2: 512 * p2 + 512],
                        g12_s[:, 128 * gp: 128 * gp + 128],
                        f12_s[:],
                    )
                au = tmp.tile([128, 1024], fp16, tag="au")
                nc.scalar.activation(au[:], q[:], Act.Exp)
                nc.vector.tensor_scalar(
                    alpha[:, 1024 * qi: 1024 * qi + 1024],
                    au[:], 0.01, 0.99, Alu.max, Alu.min)

            # constants for later phases stream in during phase A, on the
            # gpsimd DMA queue so they don't delay the phase-A inputs
            half = NG * 64
            nc.gpsimd.dma_start(tri_s[:, 0:half], tri_d[:, 0:half])
            nc.gpsimd.dma_start(tri_s[:, half:2 * half],
                                tri_d[:, half:2 * half])
            nc.gpsimd.dma_start(col6_s[:], col6_d[:])

            # ---- phase B: ln(1 - alpha) --------------------------------
            for li in range(2):
                sl = slice((NG * 128) * li, (NG * 128) * (li + 1))
                nc.scalar.activation(lnb[:, sl], alpha[:, sl], Act.Ln,
                                     bias=1.0, scale=-1.0)

            # ---- phase C: prefix matmuls, exp, alpha*W, color ----------
            # color PSUM: two chunks per pixel-half (16 cols/group;
            # chunk sizes are multiples of 4 groups, <= 32 per bank)
            ch0 = min(32, 4 * -(-NG // 8))       # e.g. NG=52 -> 28
            # a tiny final chunk keeps the post-compute tail (copy+DMA
            # after the very last matmul) as short as possible
            chunks = ((0, ch0), (ch0, NG - ch0 - 4), (NG - 4, 4))
            for (cbase, csz) in chunks:
                cp0 = cp.tile([128, csz * 16], f32, tag="c0")
                cp1 = cp.tile([128, csz * 16], f32, tag="c1")
                cps = (cp0, cp1)
                for wi in range(csz // 4):
                    w = qw.tile([128, 1024], f32, tag="qw")
                    for p4 in range(4):
                        g = cbase + 4 * wi + p4
                        nc.tensor.matmul(
                            w[:, 256 * p4: 256 * p4 + 256],
                            tri_s[:, 128 * g: 128 * g + 128],
                            lnb[:, 256 * g: 256 * g + 256],
                        )
                    g0 = cbase + 4 * wi
                    wS = tmp.tile([128, 1024], fp16, tag="ws")
                    nc.scalar.activation(wS[:], w[:], Act.Exp)
                    aw = tmp.tile([128, 1024], fp16, tag="aw")
                    nc.vector.tensor_tensor(
                        aw[:], alpha[:, 256 * g0: 256 * g0 + 1024], wS[:],
                        op=Alu.mult)
                    for p4 in range(4):
                        g = g0 + p4
                        for h in range(2):
                            nc.tensor.matmul(
                                cps[h][:, 16 * (g - cbase):
                                       16 * (g - cbase) + 16],
                                aw[:, 256 * p4 + 128 * h:
                                   256 * p4 + 128 * h + 128],
                                col6_s[:, 16 * g: 16 * g + 16],
                            )
                o0 = slice(cbase * 16, (cbase + csz) * 16)
                o1 = slice(NG * 16 + cbase * 16, NG * 16 + (cbase + csz) * 16)
                nc.vector.tensor_copy(outs[:, o0], cp0[:])
                nc.vector.tensor_copy(outs[:, o1], cp1[:])
                nc.sync.dma_start(out_d[:, o0], outs[:, o0])
                nc.gpsimd.dma_start(out_d[:, o1], outs[:, o1])

    nc.compile()
    return nc


def _ensure_ntff_hook():
    try:
        from antenv.axon_hooks import get_axon_ntff_profile_hook  # noqa: F401
        import antenv.axon_hooks as ah
    except ImportError:
        import antenv
        mod = types.ModuleType("antenv.axon_hooks")
        mod._hook = None
        def _set(h):
            mod._hook = h
        def _get():
            return mod._hook
        mod.set_axon_ntff_profile_hook = _set
        mod.get_axon_ntff_profile_hook = _get
        sys.modules["antenv.axon_hooks"] = mod
        antenv.axon_hooks = mod
        ah = mod
    if ah.get_axon_ntff_profile_hook() is None:
        from trn_agent_boot.trn_boot import _ntff_profile_via_ctypes
        ah.set_axon_ntff_profile_hook(
            _ntff_profile_via_ctypes('/opt/axon/libaxon_pjrt.so'))


# ------------------------------------------------------- numpy fallback path

def _render_numpy(pos2d, cov2d, opacity, color, depth):
    tp, tcov, topac, tcol, valid, counts, lefts, tops = _host_bin(
        pos2d, cov2d, opacity, color, depth)
    t = TILE
    gi, gj = np.meshgrid(np.arange(t), np.arange(t), indexing='ij')
    base = np.stack([gi, gj], axis=-1).astype(np.float32)
    offs = np.stack([lefts, tops], axis=-1)
    pix = base[None] + offs[:, None, None, :]
    dx = pix[:, :, :, None, 0] - tp[:, None, None, :, 0]
    dy = pix[:, :, :, None, 1] - tp[:, None, None, :, 1]
    ga = tcov[:, :, 0, 0][:, None, None, :]
    gb = tcov[:, :, 0, 1][:, None, None, :]
    gc = tcov[:, :, 1, 1][:, None, None, :]
    gdet = ga * gc - gb * gb
    quad = (gc * dx * dx - 2.0 * gb * dx * dy + ga * dy * dy) / gdet
    prob = np.exp(-0.5 * quad)
    alpha = np.clip(topac[:, None, None, :] * prob, 0.01, 0.99)
    alpha = np.where(valid[:, None, None, :], alpha, 0.0)
    weight = np.empty_like(alpha)
    weight[..., 0] = 1.0
    np.cumprod(1.0 - alpha[..., :-1], axis=-1, out=weight[..., 1:])
    aw = (alpha * weight).reshape(T, t * t, K_MAX)
    tile_img = np.matmul(aw, tcol).reshape(T, t, t, 3)
    img = tile_img.reshape(TX, TY, t, t, 3).transpose(0, 2, 1, 3, 4)
    return img.reshape(IMG_W, IMG_H, 3).astype(np.float32)


# ----------------------------------------------------------------- kernel()

def _run_device(pos2d, cov2d, opacity, color, depth):
    global _CACHED_NC, LAST_EXEC_TIME_NS, LAST_TRACE_PATH
    from concourse import bass_utils

    trace = bool(os.environ.get("GSPLAT_TRACE"))
    if trace:
        _ensure_ntff_hook()
        bass_utils.upload_artifacts = lambda tmpdir: "local://" + tmpdir

    tp, tcov, topac, tcol, valid, counts, lefts, tops = _host_bin(
        pos2d, cov2d, opacity, color, depth)
    G12s, TRIs, col6s, F12, maps = _host_pack(
        tp, tcov, topac, tcol, valid, counts, lefts, tops)

    if _CACHED_NC is None or _CACHED_NC[0] != NG:
        _CACHED_NC = (NG, _build_nc())
    nc = _CACHED_NC[1]

    in_maps = []
    for i in range(NCORES):
        in_maps.append({
            "g12": G12s[i],
            "f12": F12,
            "tri": TRIs[i],
            "col6": col6s[i],
        })
    kwargs = {}
    if trace:
        kwargs["trace"] = True
        kwargs["tmpdir"] = tempfile.mkdtemp(prefix="gsplat_trace_")
    res = bass_utils.run_bass_kernel_spmd(
        nc, in_maps, core_ids=list(range(NCORES)), **kwargs)
    if trace:
        LAST_EXEC_TIME_NS = res.exec_time_ns
        LAST_TRACE_PATH = (res.instructions_and_trace[1]
                           if res.instructions_and_trace else None)

    img_tiles = np.zeros((T, 256, 3), np.float32)
    for i in range(NCORES):
        o = res.results[i]["out"]                 # [128, NG*32]
        o = o.reshape(128, 2, NG, 16)             # [pixh, h, g, col]
        for (tg, g, ig) in maps[i]:
            t3 = o[:, :, g, 3 * ig:3 * ig + 3]    # [pixh, h, c]
            img_tiles[tg] = t3.transpose(1, 0, 2).reshape(256, 3)
    img = img_tiles.reshape(TX, TY, TILE, TILE, 3).transpose(0, 2, 1, 3, 4)
    return np.ascontiguousarray(
        img.reshape(IMG_W, IMG_H, 3), dtype=np.float32)


def kernel(pos2d, cov2d, opacity, color, depth, width=IMG_W, height=IMG_H,
           tile_length=TILE, max_per_tile=K_MAX):
    pos2d = np.asarray(pos2d, np.float32)
    cov2d = np.asarray(cov2d, np.float32)
    opacity = np.asarray(opacity, np.float32)
    color = np.asarray(color, np.float32)
    depth = np.asarray(depth, np.float32)
    ok = (int(width) == IMG_W and int(height) == IMG_H
          and int(tile_length) == TILE and int(max_per_tile) == K_MAX
          and pos2d.shape == (N_GAUSS, 2))
    if ok:
        try:
            return _run_device(pos2d, cov2d, opacity, color, depth)
        except Exception as e:  # pragma: no cover - device fallback
            print(f"kernel: device path failed ({type(e).__name__}: {e}); "
                  f"falling back to numpy", file=sys.stderr)
    return _render_numpy(pos2d, cov2d, opacity, color, depth)
